# revision 6
# baseline (speedup 1.0000x reference)
"""RWKV-4 block (TimeMix + ChannelMix) on 8 Trainium2 NeuronCores.

Sharding: data-parallel over batch (B=8 -> one batch element per core); no
collectives.  Per core, activations are kept transposed ([channel, time]) so
the WKV recurrence maps onto the DVE's hardware linear scan
(tensor_tensor_scan along the free axis, fp32 state) and channel-wise mix
coefficients become per-partition scalars.  LayerNorms run in the natural
[time, channel] layout; PE transposes move between the two.  All GEMMs run
in bf16 (full PE rate, overlapped LDWEIGHTS); WKV arithmetic in fp32.

The reference's log-space-stabilized WKV is computed here in direct form:
  lam = exp(-exp(time_decay)), eu = exp(time_first)      (host)
  A_t = lam*A_{t-1} + exp(k_t)*v_t ;  B_t likewise with exp(k_t)
  y_t = (A_{t-1} + eu*exp(k_t)*v_t) / (B_{t-1} + eu*exp(k_t))
which is exact in infinite precision; with this problem's magnitudes the
fp32 accumulators stay in range (|B| < ~5e3) so no stabilization is needed.

v2 layout: TT=1024 tiles for TimeMix/Wo (halved DVE op overheads, single
scan-carry boundary), elementwise work spread across DVE/GpSimd/ACT,
activation-table loads batched per function, Fr+sigmoid folded into the
Wo phase, Fk/Fv phase kept PE-dense, weights staged across phase
boundaries on the GpSimd DMA queue.
"""

import os
import sys
from contextlib import ExitStack

import numpy as np

for _p in ("/opt/trn_rl_repo", "/root/.axon_site/_ro/trn_rl_repo"):
    if os.path.isdir(_p) and _p not in sys.path:
        sys.path.insert(0, _p)
        break

import concourse.bass as bass
import concourse.tile as tile
from concourse import mybir, bacc
from concourse.bass_utils import run_bass_kernel_spmd
from concourse.masks import make_identity

f32 = mybir.dt.float32
bf16 = mybir.dt.bfloat16
AF = mybir.ActivationFunctionType
ALU = mybir.AluOpType
P = 128
EPS = 1e-5
ts = bass.ts

B, T, C, DA, DF = 8, 2048, 1024, 1024, 4096
N_CORES = 8


def build_rwkv_kernel(nc, T=T, C=C, DA=DA, DF=DF):
    TT = 1024            # TimeMix/Wo tile (time)
    TC = 512             # ChannelMix tile (time)
    n_t = T // TT        # 2
    n_tc = T // TC       # 4
    n_ck = C // P        # 8
    n_dk = DA // P       # 8
    n_fk = DF // P       # 32
    n_rs = TT // P       # 8 row-tiles per TT tile
    n_rc = TC // P       # 4 row-tiles per TC tile
    fph = n_fk // 2      # 16 f-tiles per half

    dma = nc.sync.dma_start

    x_d = nc.dram_tensor("x", [T, C], f32, kind="ExternalInput")
    wkT_d = nc.dram_tensor("WkT", [C, DA], bf16, kind="ExternalInput")
    wvT_d = nc.dram_tensor("WvT", [C, DA], bf16, kind="ExternalInput")
    wrT_d = nc.dram_tensor("WrT", [C, DA], bf16, kind="ExternalInput")
    woT_d = nc.dram_tensor("WoT", [DA, C], bf16, kind="ExternalInput")
    fkT_d = nc.dram_tensor("FkT", [C, DF], bf16, kind="ExternalInput")
    fvT_d = nc.dram_tensor("FvT", [DF, C], bf16, kind="ExternalInput")
    frT_d = nc.dram_tensor("FrT", [C, C], bf16, kind="ExternalInput")
    vc_d = nc.dram_tensor("vecC", [P, 9 * n_ck], f32, kind="ExternalInput")
    vd_d = nc.dram_tensor("vecD", [P, 2 * n_dk], f32, kind="ExternalInput")
    out_d = nc.dram_tensor("out", [T, C], f32, kind="ExternalOutput")

    with tile.TileContext(nc) as tc, ExitStack() as top:
        const = top.enter_context(tc.tile_pool(name="const", bufs=1))
        vc = const.tile([P, 9, n_ck], f32)
        dma(out=vc, in_=vc_d[:].rearrange("p (r a) -> p r a", a=n_ck))
        vd = const.tile([P, 2, n_dk], f32)
        dma(out=vd, in_=vd_d[:].rearrange("p (r a) -> p r a", a=n_dk))
        V = {
            "ln1_g": lambda ck: vc[:, 0, ck:ck + 1],
            "ln1_b": lambda ck: vc[:, 1, ck:ck + 1],
            "ln2_g": lambda ck: vc[:, 2, ck:ck + 1],
            "ln2_b": lambda ck: vc[:, 3, ck:ck + 1],
            "tm_k": lambda ck: vc[:, 4, ck:ck + 1],
            "tm_v": lambda ck: vc[:, 5, ck:ck + 1],
            "tm_r": lambda ck: vc[:, 6, ck:ck + 1],
            "fm_k": lambda ck: vc[:, 7, ck:ck + 1],
            "fm_r": lambda ck: vc[:, 8, ck:ck + 1],
            "lam": lambda dk: vd[:, 0, dk:dk + 1],
            "eu": lambda dk: vd[:, 1, dk:dk + 1],
        }
        ident_b = const.tile([P, P], bf16)
        make_identity(nc, ident_b)
        eps_t = const.tile([P, 1], f32)
        nc.vector.memset(eps_t, EPS)
        carryA = const.tile([P, n_dk], f32)
        carryB = const.tile([P, n_dk], f32)

        dp_rw = top.enter_context(
            tc.tile_pool(name="dp_rw", bufs=n_dk * n_t, space="DRAM"))
        dp_gk = top.enter_context(
            tc.tile_pool(name="dp_gk", bufs=n_ck * n_t, space="DRAM"))
        dp_sg = top.enter_context(
            tc.tile_pool(name="dp_sg", bufs=n_ck * n_t, space="DRAM"))
        dp_o1 = top.enter_context(
            tc.tile_pool(name="dp_o1", bufs=T // P, space="DRAM"))
        rw_dr, gk_dr, sg_dr, o1_dr = {}, {}, {}, {}

        def layernorm(pool, tagp, xr, n_chunk=2):
            # per-row mean/var -> rstd = 1/sqrt(var+eps), nbias = -mean*rstd
            st = pool.tile([P, n_chunk, 6], f32, tag=f"{tagp}_st",
                           name=f"{tagp}_st")
            cw = C // n_chunk
            for j in range(n_chunk):
                nc.vector.bn_stats(out=st[:, j, :], in_=xr[:, ts(j, cw)])
            mv = pool.tile([P, 2], f32, tag=f"{tagp}_mv", name=f"{tagp}_mv")
            nc.vector.bn_aggr(out=mv, in_=st)
            rstd = pool.tile([P, 1], f32, tag=f"{tagp}_rstd",
                             name=f"{tagp}_rstd")
            nc.scalar.activation(out=rstd, in_=mv[:, 1:2],
                                 func=AF.Abs_reciprocal_sqrt,
                                 bias=eps_t[:, 0:1])
            nbias = pool.tile([P, 1], f32, tag=f"{tagp}_nb", name=f"{tagp}_nb")
            nc.vector.tensor_tensor(out=nbias, in0=mv[:, 0:1], in1=rstd,
                                    op=ALU.mult)
            nc.vector.tensor_scalar_mul(out=nbias, in0=nbias, scalar1=-1.0)
            return rstd, nbias

        # ---------------- Phase AB1: LN1 + mix + k/v/r GEMMs + WKV --------
        with ExitStack() as ctx:
            wp = ctx.enter_context(tc.tile_pool(name="ab1_w", bufs=1))
            wk_sb = wp.tile([P, n_ck, DA], bf16)
            wv_sb = wp.tile([P, n_ck, DA], bf16)
            wr_sb = wp.tile([P, n_ck, DA], bf16)
            dma(out=wk_sb, in_=wkT_d[:].rearrange("(a p) d -> p a d", p=P))
            dma(out=wv_sb, in_=wvT_d[:].rearrange("(a p) d -> p a d", p=P))
            dma(out=wr_sb, in_=wrT_d[:].rearrange("(a p) d -> p a d", p=P))

            lnp = ctx.enter_context(tc.tile_pool(name="ab1_ln", bufs=2))
            xp = ctx.enter_context(tc.tile_pool(name="ab1_x", bufs=2))
            yp = ctx.enter_context(tc.tile_pool(name="ab1_y", bufs=1))
            hp = ctx.enter_context(tc.tile_pool(name="ab1_h", bufs=1))
            mxp = ctx.enter_context(tc.tile_pool(name="ab1_mx", bufs=1))
            dpool = ctx.enter_context(tc.tile_pool(name="ab1_d", bufs=1))
            wkv = ctx.enter_context(tc.tile_pool(name="ab1_wkv", bufs=1))
            ps_kv = ctx.enter_context(
                tc.tile_pool(name="ab1_ps_kv", bufs=4, space="PSUM"))
            ps_r = ctx.enter_context(
                tc.tile_pool(name="ab1_ps_r", bufs=2, space="PSUM"))
            ps_tr = ctx.enter_context(
                tc.tile_pool(name="ab1_ps_tr", bufs=2, space="PSUM"))

            hT = []
            for ck in range(n_ck):
                h = hp.tile([P, 1 + T], bf16, tag=f"hT{ck}", name=f"hT{ck}")
                hT.append(h)

            for it in range(n_t):
                t0 = it * TT
                ys = []
                for rs in range(n_rs):
                    row = it * n_rs + rs
                    xr = xp.tile([P, C], f32, tag="xr1", name="xr1")
                    dma(out=xr, in_=x_d[ts(row, P), :])
                    rstd, nbias = layernorm(lnp, "l1", xr)
                    y = yp.tile([P, C], bf16, tag=f"y{rs}", name=f"y{rs}")
                    nc.scalar.activation(out=y, in_=xr, func=AF.Identity,
                                         bias=nbias[:, 0:1], scale=rstd[:, 0:1])
                    ys.append(y)

                for ck in range(n_ck):
                    pt = ps_tr.tile([P, TT], bf16, tag="pt", name="pt")
                    for rs in range(n_rs):
                        nc.tensor.transpose(pt[:, ts(rs, P)],
                                            ys[rs][:, ts(ck, P)], ident_b)
                    if it == 0:
                        nc.vector.memset(hT[ck][:, 0:1], 0.0)
                    nc.scalar.activation(out=hT[ck][:, 1 + t0:1 + t0 + TT],
                                         in_=pt, func=AF.Identity,
                                         bias=V["ln1_b"](ck),
                                         scale=V["ln1_g"](ck))

                mixes = {}
                for ck in range(n_ck):
                    cur = hT[ck][:, 1 + t0:1 + t0 + TT]
                    prv = hT[ck][:, t0:t0 + TT]
                    d = dpool.tile([P, TT], bf16, tag="dmix", name="dmix")
                    nc.gpsimd.tensor_tensor(out=d, in0=cur, in1=prv,
                                            op=ALU.subtract)
                    for nm, coef in (("xk", "tm_k"), ("xv", "tm_v"),
                                     ("xr", "tm_r")):
                        td = dpool.tile([P, TT], bf16, tag="tmx", name="tmx",
                                        bufs=2)
                        nc.vector.tensor_scalar(out=td, in0=d,
                                                scalar1=V[coef](ck),
                                                scalar2=None, op0=ALU.mult)
                        mx = mxp.tile([P, TT], bf16, tag=f"{nm}{ck}",
                                      name=f"{nm}{ck}")
                        nc.gpsimd.tensor_tensor(out=mx, in0=td, in1=prv,
                                                op=ALU.add)
                        mixes[(nm, ck)] = mx

                yvs = []
                for dk in range(n_dk):
                    ek = wkv.tile([P, TT], bf16, tag="ek", name="ek", bufs=2)
                    ekv = wkv.tile([P, TT], bf16, tag="ekv", name="ekv", bufs=2)
                    for hh in range(TT // 512):
                        hs = ts(hh, 512)
                        pk = ps_kv.tile([P, 512], f32, tag="pkv", name="pkv")
                        for ck in range(n_ck):
                            nc.tensor.matmul(pk, wk_sb[:, ck, ts(dk, P)],
                                             mixes[("xk", ck)][:, hs],
                                             start=(ck == 0),
                                             stop=(ck == n_ck - 1))
                        nc.scalar.activation(out=ek[:, hs], in_=pk, func=AF.Exp)
                        pv = ps_kv.tile([P, 512], f32, tag="pkv", name="pkv")
                        for ck in range(n_ck):
                            nc.tensor.matmul(pv, wv_sb[:, ck, ts(dk, P)],
                                             mixes[("xv", ck)][:, hs],
                                             start=(ck == 0),
                                             stop=(ck == n_ck - 1))
                        nc.vector.tensor_tensor(out=ekv[:, hs], in0=ek[:, hs],
                                                in1=pv, op=ALU.mult)

                    A = wkv.tile([P, 1 + TT], f32, tag="A", name="A", bufs=1)
                    Bt = wkv.tile([P, 1 + TT], f32, tag="B", name="B", bufs=1)
                    lam_b = V["lam"](dk).to_broadcast([P, TT])
                    if it == 0:
                        nc.vector.memset(A[:, 0:1], 0.0)
                        nc.vector.memset(Bt[:, 0:1], 0.0)
                    else:
                        nc.gpsimd.tensor_copy(out=A[:, 0:1],
                                              in_=carryA[:, dk:dk + 1])
                        nc.gpsimd.tensor_copy(out=Bt[:, 0:1],
                                              in_=carryB[:, dk:dk + 1])
                    nc.vector.tensor_tensor_scan(
                        out=A[:, 1:1 + TT], data0=lam_b, data1=ekv,
                        initial=A[:, 0:1], op0=ALU.mult, op1=ALU.add)
                    nc.vector.tensor_tensor_scan(
                        out=Bt[:, 1:1 + TT], data0=lam_b, data1=ek,
                        initial=Bt[:, 0:1], op0=ALU.mult, op1=ALU.add)
                    if it != n_t - 1:
                        nc.gpsimd.tensor_copy(out=carryA[:, dk:dk + 1],
                                              in_=A[:, TT:TT + 1])
                        nc.gpsimd.tensor_copy(out=carryB[:, dk:dk + 1],
                                              in_=Bt[:, TT:TT + 1])

                    num = wkv.tile([P, TT], f32, tag="num", name="num", bufs=1)
                    nc.vector.scalar_tensor_tensor(
                        out=num, in0=ekv, scalar=V["eu"](dk), in1=A[:, 0:TT],
                        op0=ALU.mult, op1=ALU.add)
                    den = wkv.tile([P, TT], f32, tag="den", name="den", bufs=1)
                    nc.vector.scalar_tensor_tensor(
                        out=den, in0=ek, scalar=V["eu"](dk), in1=Bt[:, 0:TT],
                        op0=ALU.mult, op1=ALU.add)
                    rec = wkv.tile([P, TT], f32, tag="rec", name="rec", bufs=1)
                    nc.vector.reciprocal_approx_fast(out=rec, in_=den)
                    yv = wkv.tile([P, TT], bf16, tag=f"yv{dk}", name=f"yv{dk}",
                                  bufs=1)
                    nc.vector.tensor_tensor(out=yv, in0=num, in1=rec,
                                            op=ALU.mult)
                    yvs.append(yv)

                # r GEMM block at tile end: sigmoids batch on one act table
                for dk in range(n_dk):
                    sr = wkv.tile([P, TT], bf16, tag="sr", name="sr", bufs=1)
                    for hh in range(TT // 512):
                        hs = ts(hh, 512)
                        pr = ps_r.tile([P, 512], f32, tag="pr", name="pr")
                        for ck in range(n_ck):
                            nc.tensor.matmul(pr, wr_sb[:, ck, ts(dk, P)],
                                             mixes[("xr", ck)][:, hs],
                                             start=(ck == 0),
                                             stop=(ck == n_ck - 1))
                        nc.scalar.activation(out=sr[:, hs], in_=pr,
                                             func=AF.Sigmoid)
                    rw = wkv.tile([P, TT], bf16, tag="rw", name="rw", bufs=1)
                    nc.vector.tensor_tensor(out=rw, in0=yvs[dk], in1=sr,
                                            op=ALU.mult)
                    rd = dp_rw.tile([P, TT], bf16, tag="rw_dr", name="rw_dr")
                    dma(out=rd, in_=rw)
                    rw_dr[(dk, it)] = rd

        # Fk first half prefetch: overlaps AB2 compute, survives into CD
        fkp = top.enter_context(tc.tile_pool(name="fk_a", bufs=1))
        fka = fkp.tile([P, n_ck, DF // 2], bf16)

        # ---------------- Phase AB2: Wo + LN2 + gmix + Fr/sigmoid ---------
        with ExitStack() as ctx:
            wp2 = ctx.enter_context(tc.tile_pool(name="ab2_w", bufs=1))
            wo_sb = wp2.tile([P, n_dk, C], bf16)
            dma(out=wo_sb, in_=woT_d[:].rearrange("(a p) c -> p a c", p=P))
            fr_sb = wp2.tile([P, n_ck, C], bf16)
            nc.gpsimd.dma_start(
                out=fr_sb, in_=frT_d[:].rearrange("(a p) c -> p a c", p=P))
            nc.gpsimd.dma_start(
                out=fka,
                in_=fkT_d[:, 0:DF // 2].rearrange("(a p) d -> p a d", p=P))

            rwp = ctx.enter_context(tc.tile_pool(name="ab2_rw", bufs=1))
            xop = ctx.enter_context(tc.tile_pool(name="ab2_xo", bufs=1))
            xp2 = ctx.enter_context(tc.tile_pool(name="ab2_x", bufs=2))
            o1p = ctx.enter_context(tc.tile_pool(name="ab2_o1", bufs=2))
            y2p = ctx.enter_context(tc.tile_pool(name="ab2_y2", bufs=1))
            gp = ctx.enter_context(tc.tile_pool(name="ab2_g", bufs=1))
            g2p = ctx.enter_context(tc.tile_pool(name="ab2_g2", bufs=1))
            sgp = ctx.enter_context(tc.tile_pool(name="ab2_sg", bufs=2))
            lnp2 = ctx.enter_context(tc.tile_pool(name="ab2_ln", bufs=2))
            ps_o = ctx.enter_context(
                tc.tile_pool(name="ab2_ps_o", bufs=2, space="PSUM"))
            ps_so = ctx.enter_context(
                tc.tile_pool(name="ab2_ps_so", bufs=2, space="PSUM"))
            ps_g2 = ctx.enter_context(
                tc.tile_pool(name="ab2_ps_g2", bufs=2, space="PSUM"))
            ps_rr = ctx.enter_context(
                tc.tile_pool(name="ab2_ps_rr", bufs=2, space="PSUM"))

            gT = []
            for ck in range(n_ck):
                g = gp.tile([P, 1 + T], bf16, tag=f"gT{ck}", name=f"gT{ck}")
                gT.append(g)

            for it in range(n_t):
                t0 = it * TT
                rws = []
                for dk in range(n_dk):
                    r = rwp.tile([P, TT], bf16, tag=f"rw2_{dk}",
                                 name=f"rw2_{dk}")
                    dma(out=r, in_=rw_dr[(dk, it)])
                    rws.append(r)
                xos = []
                for ck in range(n_ck):
                    xo = xop.tile([P, TT], bf16, tag=f"xo{ck}", name=f"xo{ck}")
                    for hh in range(TT // 512):
                        hs = ts(hh, 512)
                        po = ps_o.tile([P, 512], f32, tag="po", name="po")
                        for dk in range(n_dk):
                            nc.tensor.matmul(po, wo_sb[:, dk, ts(ck, P)],
                                             rws[dk][:, hs],
                                             start=(dk == 0),
                                             stop=(dk == n_dk - 1))
                        nc.scalar.copy(out=xo[:, hs], in_=po)
                    xos.append(xo)
                y2s = []
                for rs in range(n_rs):
                    row = it * n_rs + rs
                    pso = ps_so.tile([P, C], bf16, tag="pso", name="pso")
                    for ck in range(n_ck):
                        nc.tensor.transpose(pso[:, ts(ck, P)],
                                            xos[ck][:, ts(rs, P)], ident_b)
                    xr2 = xp2.tile([P, C], f32, tag="xr2", name="xr2")
                    dma(out=xr2, in_=x_d[ts(row, P), :])
                    o1 = o1p.tile([P, C], bf16, tag="o1", name="o1")
                    nc.vector.tensor_tensor(out=o1, in0=xr2, in1=pso,
                                            op=ALU.add)
                    od = dp_o1.tile([P, C], bf16, tag="o1_dr", name="o1_dr")
                    dma(out=od, in_=o1)
                    o1_dr[row] = od
                    rstd, nbias = layernorm(lnp2, "l2", o1)
                    y2 = y2p.tile([P, C], bf16, tag=f"y2_{rs}", name=f"y2_{rs}")
                    nc.scalar.activation(out=y2, in_=o1, func=AF.Identity,
                                         bias=nbias[:, 0:1], scale=rstd[:, 0:1])
                    y2s.append(y2)
                for ck in range(n_ck):
                    pg = ps_g2.tile([P, TT], bf16, tag="pg", name="pg")
                    for rs in range(n_rs):
                        nc.tensor.transpose(pg[:, ts(rs, P)],
                                            y2s[rs][:, ts(ck, P)], ident_b)
                    if it == 0:
                        nc.vector.memset(gT[ck][:, 0:1], 0.0)
                    nc.scalar.activation(out=gT[ck][:, 1 + t0:1 + t0 + TT],
                                         in_=pg, func=AF.Identity,
                                         bias=V["ln2_b"](ck),
                                         scale=V["ln2_g"](ck))
                grs = []
                for ck in range(n_ck):
                    cur = gT[ck][:, 1 + t0:1 + t0 + TT]
                    prv = gT[ck][:, t0:t0 + TT]
                    d2 = g2p.tile([P, TT], bf16, tag="d2", name="d2", bufs=2)
                    nc.gpsimd.tensor_tensor(out=d2, in0=cur, in1=prv,
                                            op=ALU.subtract)
                    tg = g2p.tile([P, TT], bf16, tag="tgk", name="tgk", bufs=2)
                    nc.vector.tensor_scalar(out=tg, in0=d2,
                                            scalar1=V["fm_k"](ck),
                                            scalar2=None, op0=ALU.mult)
                    gk = g2p.tile([P, TT], bf16, tag="gkm", name="gkm", bufs=2)
                    nc.gpsimd.tensor_tensor(out=gk, in0=tg, in1=prv,
                                            op=ALU.add)
                    gkd = dp_gk.tile([P, TT], bf16, tag="gk_dr", name="gk_dr")
                    dma(out=gkd, in_=gk)
                    gk_dr[(ck, it)] = gkd
                    gr = g2p.tile([P, TT], bf16, tag=f"gr{ck}", name=f"gr{ck}")
                    nc.vector.scalar_tensor_tensor(
                        out=gr, in0=d2, scalar=V["fm_r"](ck), in1=prv,
                        op0=ALU.mult, op1=ALU.add)
                    grs.append(gr)
                for ck in range(n_ck):
                    sg = sgp.tile([P, TT], bf16, tag="sg", name="sg")
                    for hh in range(TT // 512):
                        hs = ts(hh, 512)
                        prr = ps_rr.tile([P, 512], f32, tag="prr", name="prr")
                        for cj in range(n_ck):
                            nc.tensor.matmul(prr, fr_sb[:, cj, ts(ck, P)],
                                             grs[cj][:, hs],
                                             start=(cj == 0),
                                             stop=(cj == n_ck - 1))
                        nc.scalar.activation(out=sg[:, hs], in_=prr,
                                             func=AF.Sigmoid)
                    sgd = dp_sg.tile([P, TT], bf16, tag="sg_dr", name="sg_dr")
                    dma(out=sgd, in_=sg)
                    sg_dr[(ck, it)] = sgd

        # ---------------- Phase CD: Fk relu^2, Fv, combine + out ----------
        with ExitStack() as ctx:
            wp3 = ctx.enter_context(tc.tile_pool(name="cd_w", bufs=1))
            fkb = wp3.tile([P, n_ck, DF // 2], bf16)
            nc.gpsimd.dma_start(
                out=fkb,
                in_=fkT_d[:, DF // 2:].rearrange("(a p) d -> p a d", p=P))
            fv_sb = wp3.tile([P, n_fk, C], bf16)
            dma(out=fv_sb, in_=fvT_d[:].rearrange("(a p) c -> p a c", p=P))

            def fk_ap(fk):
                # stationary weight slice for f-tile fk: [P(c of ck), 128(d)]
                if fk < fph:
                    return lambda ck: fka[:, ck, ts(fk, P)]
                return lambda ck: fkb[:, ck, ts(fk - fph, P)]

            gkc = ctx.enter_context(tc.tile_pool(name="cd_gk", bufs=1))
            kfp = ctx.enter_context(tc.tile_pool(name="cd_kf", bufs=1))
            cp = ctx.enter_context(tc.tile_pool(name="cd_cp", bufs=2))
            kvp = ctx.enter_context(tc.tile_pool(name="cd_kv", bufs=1))
            prodp = ctx.enter_context(tc.tile_pool(name="cd_prod", bufs=1))
            finp = ctx.enter_context(tc.tile_pool(name="cd_fin", bufs=2))
            ps_kf = ctx.enter_context(
                tc.tile_pool(name="cd_ps_kf", bufs=2, space="PSUM"))
            ps_kvp = ctx.enter_context(
                tc.tile_pool(name="cd_ps_kv", bufs=2, space="PSUM"))
            ps_sp = ctx.enter_context(
                tc.tile_pool(name="cd_ps_sp", bufs=2, space="PSUM"))

            for itc in range(n_tc):
                it2, h2 = itc // (n_tc // n_t), itc % (n_tc // n_t)
                hs2 = ts(h2, TC)
                gks = []
                for ck in range(n_ck):
                    gk = gkc.tile([P, TC], bf16, tag=f"gkc{ck}",
                                  name=f"gkc{ck}")
                    dma(out=gk, in_=gk_dr[(ck, it2)][:, hs2])
                    gks.append(gk)
                kv0 = {}
                kvs = {}
                for hf in range(2):
                    kf = kfp.tile([P, fph, TC], bf16, tag="kf", name="kf")
                    for fj in range(fph):
                        fk = hf * fph + fj
                        wap = fk_ap(fk)
                        pkf = ps_kf.tile([P, TC], f32, tag="pkf", name="pkf")
                        for ck in range(n_ck):
                            nc.tensor.matmul(pkf, wap(ck), gks[ck],
                                             start=(ck == 0),
                                             stop=(ck == n_ck - 1))
                        r1 = cp.tile([P, TC], bf16, tag="r1", name="r1")
                        nc.scalar.activation(out=r1, in_=pkf, func=AF.Relu)
                        nc.vector.tensor_tensor(out=kf[:, fj, :], in0=r1,
                                                in1=r1, op=ALU.mult)
                    for ck in range(n_ck):
                        pkv = ps_kvp.tile([P, TC], f32, tag="pkv", name="pkv")
                        for fj in range(fph):
                            nc.tensor.matmul(pkv,
                                             fv_sb[:, hf * fph + fj, ts(ck, P)],
                                             kf[:, fj, :],
                                             start=(fj == 0),
                                             stop=(fj == fph - 1))
                        if hf == 0:
                            k0 = kvp.tile([P, TC], bf16, tag=f"kv0_{ck}",
                                          name=f"kv0_{ck}")
                            nc.scalar.copy(out=k0, in_=pkv)
                            kv0[ck] = k0
                        else:
                            kv = kvp.tile([P, TC], bf16, tag=f"kv_{ck}",
                                          name=f"kv_{ck}")
                            nc.vector.tensor_tensor(out=kv, in0=kv0[ck],
                                                    in1=pkv, op=ALU.add)
                            kvs[ck] = kv
                prods = []
                for ck in range(n_ck):
                    sgt = cp.tile([P, TC], bf16, tag="sgl", name="sgl")
                    dma(out=sgt, in_=sg_dr[(ck, it2)][:, hs2])
                    prod = prodp.tile([P, TC], bf16, tag=f"prod{ck}",
                                      name=f"prod{ck}")
                    nc.vector.tensor_tensor(out=prod, in0=sgt, in1=kvs[ck],
                                            op=ALU.mult)
                    prods.append(prod)
                for rs in range(n_rc):
                    row = itc * n_rc + rs
                    psp = ps_sp.tile([P, C], bf16, tag="psp", name="psp")
                    for ck in range(n_ck):
                        nc.tensor.transpose(psp[:, ts(ck, P)],
                                            prods[ck][:, ts(rs, P)], ident_b)
                    o1t = finp.tile([P, C], bf16, tag="o1c", name="o1c")
                    dma(out=o1t, in_=o1_dr[row])
                    fin = finp.tile([P, C], f32, tag="fin", name="fin")
                    nc.vector.tensor_tensor(out=fin, in0=o1t, in1=psp,
                                            op=ALU.add)
                    dma(out=out_d[ts(row, P), :], in_=fin)
    return nc


def make_host_inputs(inputs, C=C, DA=DA):
    import ml_dtypes
    bf = ml_dtypes.bfloat16
    a = np.asarray
    n_ck = C // P
    n_dk = DA // P
    vecC = np.stack([
        a(inputs["ln1_g"]), a(inputs["ln1_b"]),
        a(inputs["ln2_g"]), a(inputs["ln2_b"]),
        a(inputs["tm_k"]), a(inputs["tm_v"]), a(inputs["tm_r"]),
        a(inputs["fm_k"]), a(inputs["fm_r"]),
    ]).astype(np.float32)
    vecD = np.stack([
        np.exp(-np.exp(a(inputs["time_decay"]).astype(np.float64))),
        np.exp(a(inputs["time_first"]).astype(np.float64)),
    ]).astype(np.float32)
    vecC_pm = np.ascontiguousarray(
        vecC.reshape(9, n_ck, P).transpose(2, 0, 1).reshape(P, 9 * n_ck))
    vecD_pm = np.ascontiguousarray(
        vecD.reshape(2, n_dk, P).transpose(2, 0, 1).reshape(P, 2 * n_dk))
    t = lambda w: np.ascontiguousarray(a(w).astype(np.float32).T.astype(bf))
    return {
        "WkT": t(inputs["Wk"]), "WvT": t(inputs["Wv"]), "WrT": t(inputs["Wr"]),
        "WoT": t(inputs["Wo"]), "FkT": t(inputs["Fk"]), "FvT": t(inputs["Fv"]),
        "FrT": t(inputs["Fr"]), "vecC": vecC_pm, "vecD": vecD_pm,
    }


_NC = None
LAST_EXEC_NS = None
LAST_RESULTS = None


def _get_nc():
    global _NC
    if _NC is None:
        nc = bacc.Bacc("TRN2", target_bir_lowering=False, debug=False)
        build_rwkv_kernel(nc)
        nc.compile()
        _NC = nc
    return _NC


def _maybe_install_trace_hook():
    """Best-effort NTFF profile hook shim (used when RWKV_BASS_TRACE=1)."""
    import types
    try:
        from antenv.axon_hooks import get_axon_ntff_profile_hook  # noqa: F401
        return True
    except ImportError:
        pass
    try:
        if "/root/.axon_site" not in sys.path and os.path.isdir("/root/.axon_site"):
            sys.path.insert(0, "/root/.axon_site")
        from trn_agent_boot.trn_boot import _ntff_profile_via_ctypes
        import antenv
        hookmod = types.ModuleType("antenv.axon_hooks")
        hookmod._hook = _ntff_profile_via_ctypes("/opt/axon/libaxon_pjrt.so")
        hookmod.set_axon_ntff_profile_hook = lambda h: setattr(hookmod, "_hook", h)
        hookmod.get_axon_ntff_profile_hook = lambda: hookmod._hook
        sys.modules["antenv.axon_hooks"] = hookmod
        antenv.axon_hooks = hookmod
        return True
    except Exception:
        return False


def kernel(**inputs):
    global LAST_EXEC_NS
    x = np.asarray(inputs["x"], dtype=np.float32)
    assert x.shape == (B, T, C), x.shape
    nc = _get_nc()
    shared = make_host_inputs(inputs)
    in_maps = [dict(shared, x=np.ascontiguousarray(x[i])) for i in range(N_CORES)]
    trace = os.environ.get("RWKV_BASS_TRACE", "") == "1"
    if trace:
        trace = _maybe_install_trace_hook()
    res = run_bass_kernel_spmd(nc, in_maps, list(range(N_CORES)), trace=trace)
    global LAST_RESULTS
    LAST_RESULTS = res
    LAST_EXEC_NS = res.exec_time_ns
    out = np.stack([res.results[i]["out"] for i in range(N_CORES)])
    return out.astype(np.float32)


# revision 11
# speedup vs baseline: 1.0427x; 1.0427x over previous
"""RWKV-4 block (TimeMix + ChannelMix) on 8 Trainium2 NeuronCores.

Sharding: data-parallel over batch (B=8 -> one batch element per core); no
collectives.  Per core, activations are kept transposed ([channel, time]) so
the WKV recurrence maps onto the DVE's hardware linear scan
(tensor_tensor_scan along the free axis, fp32 state) and channel-wise mix
coefficients become per-partition scalars.  LayerNorms run in the natural
[time, channel] layout; PE transposes move between the two.  All GEMMs run
in bf16 (full PE rate, overlapped LDWEIGHTS); WKV state in fp32.

WKV in direct form with the receptance sigmoid folded into the division:
  lam = exp(-exp(time_decay)), eu = exp(time_first)      (host)
  A_t = lam*A_{t-1} + exp(k_t)*v_t ;  B_t likewise with exp(k_t)
  y_t*sr_t = (A_{t-1} + eu*ek_t*v_t) / ((B_{t-1} + eu*ek_t)*(1+exp(-r_t)))
Wr is negated host-side so exp(-r) is a plain Exp on the ACT engine; the
whole phase then needs only the exp activation table (plus one rsqrt load
for LayerNorm), and the scans run over the full T=2048 with no carries.

v3 layout: LN hoisted for all of T, mixes computed once at [P, 2048]
(bf16), per-dk GEMM chunks feed full-length scans, elementwise work spread
DVE/GpSimd/ACT (GpSimd only does bf16/f32xbf16 tensor_tensor - other op
shapes miscompile), Fr+sigmoid folded into the Wo phase, Fk/Fv phase kept
PE-dense, weights staged across phase boundaries on the GpSimd DMA queue.
"""

import os
import sys
from contextlib import ExitStack

import numpy as np

for _p in ("/opt/trn_rl_repo", "/root/.axon_site/_ro/trn_rl_repo"):
    if os.path.isdir(_p) and _p not in sys.path:
        sys.path.insert(0, _p)
        break

import concourse.bass as bass
import concourse.tile as tile
from concourse import mybir, bacc
from concourse.bass_utils import run_bass_kernel_spmd
from concourse.masks import make_identity

f32 = mybir.dt.float32
bf16 = mybir.dt.bfloat16
AF = mybir.ActivationFunctionType
ALU = mybir.AluOpType
P = 128
EPS = 1e-5
ts = bass.ts

B, T, C, DA, DF = 8, 2048, 1024, 1024, 4096
N_CORES = 8


def build_rwkv_kernel(nc, T=T, C=C, DA=DA, DF=DF):
    TT = 1024            # Wo/LN2 tile (time)
    TC = 512             # ChannelMix tile (time)
    n_t = T // TT        # 2
    n_tc = T // TC       # 4
    n_ck = C // P        # 8
    n_dk = DA // P       # 8
    n_fk = DF // P       # 32
    n_rs = TT // P       # 8 row-tiles per TT tile
    n_rc = TC // P       # 4 row-tiles per TC tile
    n_q = T // 512       # 4 GEMM chunks over full T
    fph = n_fk // 2      # 16 f-tiles per half

    dma = nc.sync.dma_start

    x_d = nc.dram_tensor("x", [T, C], f32, kind="ExternalInput")
    wkT_d = nc.dram_tensor("WkT", [C, DA], bf16, kind="ExternalInput")
    wvT_d = nc.dram_tensor("WvT", [C, DA], bf16, kind="ExternalInput")
    wrT_d = nc.dram_tensor("WrT", [C, DA], bf16, kind="ExternalInput")
    woT_d = nc.dram_tensor("WoT", [DA, C], bf16, kind="ExternalInput")
    fkT_d = nc.dram_tensor("FkT", [C, DF], bf16, kind="ExternalInput")
    fvT_d = nc.dram_tensor("FvT", [DF, C], bf16, kind="ExternalInput")
    frT_d = nc.dram_tensor("FrT", [C, C], bf16, kind="ExternalInput")
    vc_d = nc.dram_tensor("vecC", [P, 9 * n_ck], f32, kind="ExternalInput")
    vd_d = nc.dram_tensor("vecD", [P, 2 * n_dk], f32, kind="ExternalInput")
    out_d = nc.dram_tensor("out", [T, C], f32, kind="ExternalOutput")

    with tile.TileContext(nc) as tc, ExitStack() as top:
        const = top.enter_context(tc.tile_pool(name="const", bufs=1))
        vc = const.tile([P, 9, n_ck], f32)
        dma(out=vc, in_=vc_d[:].rearrange("p (r a) -> p r a", a=n_ck))
        vd = const.tile([P, 2, n_dk], f32)
        dma(out=vd, in_=vd_d[:].rearrange("p (r a) -> p r a", a=n_dk))
        V = {
            "ln1_g": lambda ck: vc[:, 0, ck:ck + 1],
            "ln1_b": lambda ck: vc[:, 1, ck:ck + 1],
            "ln2_g": lambda ck: vc[:, 2, ck:ck + 1],
            "ln2_b": lambda ck: vc[:, 3, ck:ck + 1],
            "tm_k": lambda ck: vc[:, 4, ck:ck + 1],
            "tm_v": lambda ck: vc[:, 5, ck:ck + 1],
            "tm_r": lambda ck: vc[:, 6, ck:ck + 1],
            "fm_k": lambda ck: vc[:, 7, ck:ck + 1],
            "fm_r": lambda ck: vc[:, 8, ck:ck + 1],
            "lam": lambda dk: vd[:, 0, dk:dk + 1],
            "eu": lambda dk: vd[:, 1, dk:dk + 1],
        }
        ident_b = const.tile([P, P], bf16)
        make_identity(nc, ident_b)
        eps_t = const.tile([P, 1], f32)
        nc.vector.memset(eps_t, EPS)
        one_t = const.tile([P, 1], f32)
        nc.vector.memset(one_t, 1.0)

        dp_rw = top.enter_context(
            tc.tile_pool(name="dp_rw", bufs=n_dk, space="DRAM"))
        dp_gk = top.enter_context(
            tc.tile_pool(name="dp_gk", bufs=n_ck * n_t, space="DRAM"))
        dp_sg = top.enter_context(
            tc.tile_pool(name="dp_sg", bufs=n_ck * n_t, space="DRAM"))
        dp_o1 = top.enter_context(
            tc.tile_pool(name="dp_o1", bufs=T // P, space="DRAM"))
        rw_dr, gk_dr, sg_dr, o1_dr = {}, {}, {}, {}

        def layernorm(pool, tagp, xr, n_chunk=2):
            st = pool.tile([P, n_chunk, 6], f32, tag=f"{tagp}_st",
                           name=f"{tagp}_st")
            cw = C // n_chunk
            for j in range(n_chunk):
                nc.vector.bn_stats(out=st[:, j, :], in_=xr[:, ts(j, cw)])
            mv = pool.tile([P, 2], f32, tag=f"{tagp}_mv", name=f"{tagp}_mv")
            nc.vector.bn_aggr(out=mv, in_=st)
            rstd = pool.tile([P, 1], f32, tag=f"{tagp}_rstd",
                             name=f"{tagp}_rstd")
            nc.scalar.activation(out=rstd, in_=mv[:, 1:2],
                                 func=AF.Abs_reciprocal_sqrt,
                                 bias=eps_t[:, 0:1])
            nbias = pool.tile([P, 1], f32, tag=f"{tagp}_nb", name=f"{tagp}_nb")
            nc.vector.tensor_tensor(out=nbias, in0=mv[:, 0:1], in1=rstd,
                                    op=ALU.mult)
            nc.vector.tensor_scalar_mul(out=nbias, in0=nbias, scalar1=-1.0)
            return rstd, nbias

        # ---------------- Phase AB1: LN1 + mix + k/v/r GEMMs + WKV --------
        with ExitStack() as ctx:
            wp = ctx.enter_context(tc.tile_pool(name="ab1_w", bufs=1))
            wk_sb = wp.tile([P, n_ck, DA], bf16)
            wv_sb = wp.tile([P, n_ck, DA], bf16)
            wr_sb = wp.tile([P, n_ck, DA], bf16)
            dma(out=wk_sb, in_=wkT_d[:].rearrange("(a p) d -> p a d", p=P))
            dma(out=wv_sb, in_=wvT_d[:].rearrange("(a p) d -> p a d", p=P))
            dma(out=wr_sb, in_=wrT_d[:].rearrange("(a p) d -> p a d", p=P))

            mxp = ctx.enter_context(tc.tile_pool(name="ab1_mx", bufs=1))

            # --- LN stage over the whole sequence ---
            lhctx = ExitStack()
            hp = lhctx.enter_context(tc.tile_pool(name="ab1_h", bufs=1))
            hT = [hp.tile([P, 1 + T], bf16, tag=f"hT{ck}", name=f"hT{ck}")
                  for ck in range(n_ck)]
            with ExitStack() as lctx:
                lnp = lctx.enter_context(tc.tile_pool(name="ab1_ln", bufs=2))
                xp = lctx.enter_context(tc.tile_pool(name="ab1_x", bufs=2))
                yp = lctx.enter_context(tc.tile_pool(name="ab1_y", bufs=1))
                ps_tr = lctx.enter_context(
                    tc.tile_pool(name="ab1_ps_tr", bufs=2, space="PSUM"))
                for ck in range(n_ck):
                    nc.vector.memset(hT[ck][:, 0:1], 0.0)
                for half in range(n_t):
                    t0 = half * TT
                    ys = []
                    for rs in range(n_rs):
                        row = half * n_rs + rs
                        xr = xp.tile([P, C], f32, tag="xr1", name="xr1")
                        dma(out=xr, in_=x_d[ts(row, P), :])
                        rstd, nbias = layernorm(lnp, "l1", xr)
                        y = yp.tile([P, C], bf16, tag=f"y{rs}", name=f"y{rs}")
                        nc.scalar.activation(out=y, in_=xr, func=AF.Identity,
                                             bias=nbias[:, 0:1],
                                             scale=rstd[:, 0:1])
                        ys.append(y)
                    for ck in range(n_ck):
                        pt = ps_tr.tile([P, TT], bf16, tag="pt", name="pt")
                        for rs in range(n_rs):
                            nc.tensor.transpose(pt[:, ts(rs, P)],
                                                ys[rs][:, ts(ck, P)], ident_b)
                        nc.scalar.activation(
                            out=hT[ck][:, 1 + t0:1 + t0 + TT], in_=pt,
                            func=AF.Identity, bias=V["ln1_b"](ck),
                            scale=V["ln1_g"](ck))

            # --- mix stage: full-T xk/xv/xr ---
            mixes = {}
            with ExitStack() as mctx:
                dpool = mctx.enter_context(tc.tile_pool(name="ab1_d", bufs=1))
                for ck in range(n_ck):
                    cur = hT[ck][:, 1:1 + T]
                    prv = hT[ck][:, 0:T]
                    d = dpool.tile([P, T], bf16, tag="dmix", name="dmix",
                                   bufs=2)
                    nc.gpsimd.tensor_tensor(out=d, in0=cur, in1=prv,
                                            op=ALU.subtract)
                    for nm, coef in (("xk", "tm_k"), ("xv", "tm_v"),
                                     ("xr", "tm_r")):
                        td = dpool.tile([P, T], bf16, tag="tmx", name="tmx",
                                        bufs=2)
                        nc.scalar.activation(out=td, in_=d, func=AF.Identity,
                                             scale=V[coef](ck))
                        mx = mxp.tile([P, T], bf16, tag=f"{nm}{ck}",
                                      name=f"{nm}{ck}")
                        nc.vector.tensor_tensor(out=mx, in0=td, in1=prv,
                                                op=ALU.add)
                        mixes[(nm, ck)] = mx
            lhctx.close()  # hT dead: free its 33KB for the WKV pool

            # --- WKV stage: per dk over full T ---
            wkv = ctx.enter_context(tc.tile_pool(name="ab1_wkv", bufs=1))
            ps_kv = ctx.enter_context(
                tc.tile_pool(name="ab1_ps_kv", bufs=4, space="PSUM"))
            ps_r = ctx.enter_context(
                tc.tile_pool(name="ab1_ps_r", bufs=2, space="PSUM"))
            for dk in range(n_dk):
                ek = wkv.tile([P, T], bf16, tag="ek", name="ek", bufs=2)
                em = wkv.tile([P, T], bf16, tag="em", name="em", bufs=1)
                vsb = wkv.tile([P, T], bf16, tag="vsb", name="vsb", bufs=1)
                ekv = wkv.tile([P, T], bf16, tag="ekv", name="ekv", bufs=1)
                for q in range(n_q):
                    qs = ts(q, 512)
                    pk = ps_kv.tile([P, 512], f32, tag="pkv", name="pkv")
                    for ck in range(n_ck):
                        nc.tensor.matmul(pk, wk_sb[:, ck, ts(dk, P)],
                                         mixes[("xk", ck)][:, qs],
                                         start=(ck == 0),
                                         stop=(ck == n_ck - 1))
                    nc.scalar.activation(out=ek[:, qs], in_=pk, func=AF.Exp)
                    pv = ps_kv.tile([P, 512], f32, tag="pkv", name="pkv")
                    for ck in range(n_ck):
                        nc.tensor.matmul(pv, wv_sb[:, ck, ts(dk, P)],
                                         mixes[("xv", ck)][:, qs],
                                         start=(ck == 0),
                                         stop=(ck == n_ck - 1))
                    nc.scalar.copy(out=vsb[:, qs], in_=pv)
                    nc.gpsimd.tensor_tensor(out=ekv[:, qs], in0=ek[:, qs],
                                            in1=vsb[:, qs], op=ALU.mult)
                    pr = ps_r.tile([P, 512], f32, tag="pr", name="pr")
                    for ck in range(n_ck):
                        # Wr is negated host-side: pr = -r
                        nc.tensor.matmul(pr, wr_sb[:, ck, ts(dk, P)],
                                         mixes[("xr", ck)][:, qs],
                                         start=(ck == 0),
                                         stop=(ck == n_ck - 1))
                    nc.scalar.activation(out=em[:, qs], in_=pr, func=AF.Exp)
                # ep = 1 + exp(-r)  (bf16; ACT adds the per-partition 1)
                ep = wkv.tile([P, T], bf16, tag="ep", name="ep", bufs=1)
                nc.scalar.activation(out=ep, in_=em, func=AF.Identity,
                                     bias=one_t[:, 0:1])

                A = wkv.tile([P, 1 + T], f32, tag="A", name="A", bufs=1)
                Bt = wkv.tile([P, 1 + T], f32, tag="B", name="B", bufs=1)
                lam_b = V["lam"](dk).to_broadcast([P, T])
                nc.vector.memset(A[:, 0:1], 0.0)
                nc.vector.memset(Bt[:, 0:1], 0.0)
                nc.vector.tensor_tensor_scan(
                    out=A[:, 1:1 + T], data0=lam_b, data1=ekv,
                    initial=A[:, 0:1], op0=ALU.mult, op1=ALU.add)
                nc.vector.tensor_tensor_scan(
                    out=Bt[:, 1:1 + T], data0=lam_b, data1=ek,
                    initial=Bt[:, 0:1], op0=ALU.mult, op1=ALU.add)

                rw = wkv.tile([P, T], bf16, tag="rw", name="rw", bufs=1)
                for h in range(n_t):
                    hs = ts(h, TT)
                    num = wkv.tile([P, TT], bf16, tag="num", name="num",
                                   bufs=1)
                    nc.vector.scalar_tensor_tensor(
                        out=num, in0=ekv[:, hs], scalar=V["eu"](dk),
                        in1=A[:, h * TT:h * TT + TT],
                        op0=ALU.mult, op1=ALU.add)
                    den = wkv.tile([P, TT], f32, tag="den", name="den",
                                   bufs=1)
                    nc.vector.scalar_tensor_tensor(
                        out=den, in0=ek[:, hs], scalar=V["eu"](dk),
                        in1=Bt[:, h * TT:h * TT + TT],
                        op0=ALU.mult, op1=ALU.add)
                    den2 = wkv.tile([P, TT], f32, tag="den2", name="den2",
                                    bufs=1)
                    nc.gpsimd.tensor_tensor(out=den2, in0=den, in1=ep[:, hs],
                                            op=ALU.mult)
                    rec = wkv.tile([P, TT], f32, tag="rec", name="rec",
                                   bufs=2)
                    nc.vector.reciprocal_approx_fast(out=rec, in_=den2)
                    nc.vector.tensor_tensor(out=rw[:, hs], in0=num, in1=rec,
                                            op=ALU.mult)
                rd = dp_rw.tile([P, T], bf16, tag="rw_dr", name="rw_dr")
                dma(out=rd, in_=rw)
                rw_dr[dk] = rd

        # Fk first half prefetch: overlaps AB2 compute, survives into CD
        fkp = top.enter_context(tc.tile_pool(name="fk_a", bufs=1))
        fka = fkp.tile([P, n_ck, DF // 2], bf16)

        # ---------------- Phase AB2: Wo + LN2 + gmix + Fr/sigmoid ---------
        with ExitStack() as ctx:
            wp2 = ctx.enter_context(tc.tile_pool(name="ab2_w", bufs=1))
            wo_sb = wp2.tile([P, n_dk, C], bf16)
            dma(out=wo_sb, in_=woT_d[:].rearrange("(a p) c -> p a c", p=P))
            fr_sb = wp2.tile([P, n_ck, C], bf16)
            nc.gpsimd.dma_start(
                out=fr_sb, in_=frT_d[:].rearrange("(a p) c -> p a c", p=P))
            nc.gpsimd.dma_start(
                out=fka,
                in_=fkT_d[:, 0:DF // 2].rearrange("(a p) d -> p a d", p=P))

            rwp = ctx.enter_context(tc.tile_pool(name="ab2_rw", bufs=1))
            xop = ctx.enter_context(tc.tile_pool(name="ab2_xo", bufs=1))
            xp2 = ctx.enter_context(tc.tile_pool(name="ab2_x", bufs=2))
            o1p = ctx.enter_context(tc.tile_pool(name="ab2_o1", bufs=2))
            y2p = ctx.enter_context(tc.tile_pool(name="ab2_y2", bufs=1))
            gp = ctx.enter_context(tc.tile_pool(name="ab2_g", bufs=1))
            g2p = ctx.enter_context(tc.tile_pool(name="ab2_g2", bufs=1))
            sgp = ctx.enter_context(tc.tile_pool(name="ab2_sg", bufs=2))
            lnp2 = ctx.enter_context(tc.tile_pool(name="ab2_ln", bufs=2))
            ps_o = ctx.enter_context(
                tc.tile_pool(name="ab2_ps_o", bufs=2, space="PSUM"))
            ps_so = ctx.enter_context(
                tc.tile_pool(name="ab2_ps_so", bufs=2, space="PSUM"))
            ps_g2 = ctx.enter_context(
                tc.tile_pool(name="ab2_ps_g2", bufs=2, space="PSUM"))
            ps_rr = ctx.enter_context(
                tc.tile_pool(name="ab2_ps_rr", bufs=2, space="PSUM"))

            gT = [gp.tile([P, 1 + T], bf16, tag=f"gT{ck}", name=f"gT{ck}")
                  for ck in range(n_ck)]

            for it in range(n_t):
                t0 = it * TT
                rws = []
                for dk in range(n_dk):
                    r = rwp.tile([P, TT], bf16, tag=f"rw2_{dk}",
                                 name=f"rw2_{dk}")
                    dma(out=r, in_=rw_dr[dk][:, ts(it, TT)])
                    rws.append(r)
                xos = []
                for ck in range(n_ck):
                    xo = xop.tile([P, TT], bf16, tag=f"xo{ck}", name=f"xo{ck}")
                    for hh in range(TT // 512):
                        hs = ts(hh, 512)
                        po = ps_o.tile([P, 512], f32, tag="po", name="po")
                        for dk in range(n_dk):
                            nc.tensor.matmul(po, wo_sb[:, dk, ts(ck, P)],
                                             rws[dk][:, hs],
                                             start=(dk == 0),
                                             stop=(dk == n_dk - 1))
                        nc.scalar.copy(out=xo[:, hs], in_=po)
                    xos.append(xo)
                y2s = []
                for rs in range(n_rs):
                    row = it * n_rs + rs
                    pso = ps_so.tile([P, C], bf16, tag="pso", name="pso")
                    for ck in range(n_ck):
                        nc.tensor.transpose(pso[:, ts(ck, P)],
                                            xos[ck][:, ts(rs, P)], ident_b)
                    xr2 = xp2.tile([P, C], f32, tag="xr2", name="xr2")
                    dma(out=xr2, in_=x_d[ts(row, P), :])
                    o1 = o1p.tile([P, C], bf16, tag="o1", name="o1")
                    nc.vector.tensor_tensor(out=o1, in0=xr2, in1=pso,
                                            op=ALU.add)
                    od = dp_o1.tile([P, C], bf16, tag="o1_dr", name="o1_dr")
                    dma(out=od, in_=o1)
                    o1_dr[row] = od
                    rstd, nbias = layernorm(lnp2, "l2", o1)
                    y2 = y2p.tile([P, C], bf16, tag=f"y2_{rs}",
                                  name=f"y2_{rs}")
                    nc.scalar.activation(out=y2, in_=o1, func=AF.Identity,
                                         bias=nbias[:, 0:1], scale=rstd[:, 0:1])
                    y2s.append(y2)
                for ck in range(n_ck):
                    pg = ps_g2.tile([P, TT], bf16, tag="pg", name="pg")
                    for rs in range(n_rs):
                        nc.tensor.transpose(pg[:, ts(rs, P)],
                                            y2s[rs][:, ts(ck, P)], ident_b)
                    if it == 0:
                        nc.vector.memset(gT[ck][:, 0:1], 0.0)
                    nc.scalar.activation(out=gT[ck][:, 1 + t0:1 + t0 + TT],
                                         in_=pg, func=AF.Identity,
                                         bias=V["ln2_b"](ck),
                                         scale=V["ln2_g"](ck))
                grs = []
                for ck in range(n_ck):
                    cur = gT[ck][:, 1 + t0:1 + t0 + TT]
                    prv = gT[ck][:, t0:t0 + TT]
                    d2 = g2p.tile([P, TT], bf16, tag="d2", name="d2", bufs=2)
                    nc.vector.tensor_tensor(out=d2, in0=cur, in1=prv,
                                            op=ALU.subtract)
                    tg = g2p.tile([P, TT], bf16, tag="tgk", name="tgk", bufs=2)
                    nc.scalar.activation(out=tg, in_=d2, func=AF.Identity,
                                         scale=V["fm_k"](ck))
                    gk = g2p.tile([P, TT], bf16, tag="gkm", name="gkm", bufs=2)
                    nc.vector.tensor_tensor(out=gk, in0=tg, in1=prv,
                                            op=ALU.add)
                    gkd = dp_gk.tile([P, TT], bf16, tag="gk_dr", name="gk_dr")
                    dma(out=gkd, in_=gk)
                    gk_dr[(ck, it)] = gkd
                    gr = g2p.tile([P, TT], bf16, tag=f"gr{ck}", name=f"gr{ck}")
                    nc.vector.scalar_tensor_tensor(
                        out=gr, in0=d2, scalar=V["fm_r"](ck), in1=prv,
                        op0=ALU.mult, op1=ALU.add)
                    grs.append(gr)
                for ck in range(n_ck):
                    sg = sgp.tile([P, TT], bf16, tag="sg", name="sg")
                    for hh in range(TT // 512):
                        hs = ts(hh, 512)
                        prr = ps_rr.tile([P, 512], f32, tag="prr", name="prr")
                        for cj in range(n_ck):
                            nc.tensor.matmul(prr, fr_sb[:, cj, ts(ck, P)],
                                             grs[cj][:, hs],
                                             start=(cj == 0),
                                             stop=(cj == n_ck - 1))
                        nc.scalar.activation(out=sg[:, hs], in_=prr,
                                             func=AF.Sigmoid)
                    sgd = dp_sg.tile([P, TT], bf16, tag="sg_dr", name="sg_dr")
                    dma(out=sgd, in_=sg)
                    sg_dr[(ck, it)] = sgd

        # ---------------- Phase CD: Fk relu^2, Fv, combine + out ----------
        with ExitStack() as ctx:
            wp3 = ctx.enter_context(tc.tile_pool(name="cd_w", bufs=1))
            fkb = wp3.tile([P, n_ck, DF // 2], bf16)
            nc.gpsimd.dma_start(
                out=fkb,
                in_=fkT_d[:, DF // 2:].rearrange("(a p) d -> p a d", p=P))
            fv_sb = wp3.tile([P, n_fk, C], bf16)
            dma(out=fv_sb, in_=fvT_d[:].rearrange("(a p) c -> p a c", p=P))

            def fk_ap(fk):
                if fk < fph:
                    return lambda ck: fka[:, ck, ts(fk, P)]
                return lambda ck: fkb[:, ck, ts(fk - fph, P)]

            gkc = ctx.enter_context(tc.tile_pool(name="cd_gk", bufs=1))
            kfp = ctx.enter_context(tc.tile_pool(name="cd_kf", bufs=1))
            cp = ctx.enter_context(tc.tile_pool(name="cd_cp", bufs=2))
            kvp = ctx.enter_context(tc.tile_pool(name="cd_kv", bufs=1))
            prodp = ctx.enter_context(tc.tile_pool(name="cd_prod", bufs=1))
            finp = ctx.enter_context(tc.tile_pool(name="cd_fin", bufs=2))
            ps_kf = ctx.enter_context(
                tc.tile_pool(name="cd_ps_kf", bufs=2, space="PSUM"))
            ps_kvp = ctx.enter_context(
                tc.tile_pool(name="cd_ps_kv", bufs=2, space="PSUM"))
            ps_sp = ctx.enter_context(
                tc.tile_pool(name="cd_ps_sp", bufs=2, space="PSUM"))

            for itc in range(n_tc):
                it2, h2 = itc // (n_tc // n_t), itc % (n_tc // n_t)
                hs2 = ts(h2, TC)
                gks = []
                for ck in range(n_ck):
                    gk = gkc.tile([P, TC], bf16, tag=f"gkc{ck}",
                                  name=f"gkc{ck}")
                    dma(out=gk, in_=gk_dr[(ck, it2)][:, hs2])
                    gks.append(gk)
                kv0 = {}
                kvs = {}
                for hf in range(2):
                    kf = kfp.tile([P, fph, TC], bf16, tag="kf", name="kf")
                    for fj in range(fph):
                        fk = hf * fph + fj
                        wap = fk_ap(fk)
                        pkf = ps_kf.tile([P, TC], f32, tag="pkf", name="pkf")
                        for ck in range(n_ck):
                            nc.tensor.matmul(pkf, wap(ck), gks[ck],
                                             start=(ck == 0),
                                             stop=(ck == n_ck - 1))
                        r1 = cp.tile([P, TC], bf16, tag="r1", name="r1")
                        nc.scalar.activation(out=r1, in_=pkf, func=AF.Relu)
                        nc.vector.tensor_tensor(out=kf[:, fj, :], in0=r1,
                                                in1=r1, op=ALU.mult)
                    for ck in range(n_ck):
                        pkv = ps_kvp.tile([P, TC], f32, tag="pkv", name="pkv")
                        for fj in range(fph):
                            nc.tensor.matmul(pkv,
                                             fv_sb[:, hf * fph + fj, ts(ck, P)],
                                             kf[:, fj, :],
                                             start=(fj == 0),
                                             stop=(fj == fph - 1))
                        if hf == 0:
                            k0 = kvp.tile([P, TC], bf16, tag=f"kv0_{ck}",
                                          name=f"kv0_{ck}")
                            nc.scalar.copy(out=k0, in_=pkv)
                            kv0[ck] = k0
                        else:
                            kv = kvp.tile([P, TC], bf16, tag=f"kv_{ck}",
                                          name=f"kv_{ck}")
                            nc.vector.tensor_tensor(out=kv, in0=kv0[ck],
                                                    in1=pkv, op=ALU.add)
                            kvs[ck] = kv
                prods = []
                for ck in range(n_ck):
                    sgt = cp.tile([P, TC], bf16, tag="sgl", name="sgl")
                    dma(out=sgt, in_=sg_dr[(ck, it2)][:, hs2])
                    prod = prodp.tile([P, TC], bf16, tag=f"prod{ck}",
                                      name=f"prod{ck}")
                    nc.vector.tensor_tensor(out=prod, in0=sgt, in1=kvs[ck],
                                            op=ALU.mult)
                    prods.append(prod)
                for rs in range(n_rc):
                    row = itc * n_rc + rs
                    psp = ps_sp.tile([P, C], bf16, tag="psp", name="psp")
                    for ck in range(n_ck):
                        nc.tensor.transpose(psp[:, ts(ck, P)],
                                            prods[ck][:, ts(rs, P)], ident_b)
                    o1t = finp.tile([P, C], bf16, tag="o1c", name="o1c")
                    dma(out=o1t, in_=o1_dr[row])
                    fin = finp.tile([P, C], f32, tag="fin", name="fin")
                    nc.vector.tensor_tensor(out=fin, in0=o1t, in1=psp,
                                            op=ALU.add)
                    dma(out=out_d[ts(row, P), :], in_=fin)
    return nc


def make_host_inputs(inputs, C=C, DA=DA):
    import ml_dtypes
    bf = ml_dtypes.bfloat16
    a = np.asarray
    n_ck = C // P
    n_dk = DA // P
    vecC = np.stack([
        a(inputs["ln1_g"]), a(inputs["ln1_b"]),
        a(inputs["ln2_g"]), a(inputs["ln2_b"]),
        a(inputs["tm_k"]), a(inputs["tm_v"]), a(inputs["tm_r"]),
        a(inputs["fm_k"]), a(inputs["fm_r"]),
    ]).astype(np.float32)
    vecD = np.stack([
        np.exp(-np.exp(a(inputs["time_decay"]).astype(np.float64))),
        np.exp(a(inputs["time_first"]).astype(np.float64)),
    ]).astype(np.float32)
    vecC_pm = np.ascontiguousarray(
        vecC.reshape(9, n_ck, P).transpose(2, 0, 1).reshape(P, 9 * n_ck))
    vecD_pm = np.ascontiguousarray(
        vecD.reshape(2, n_dk, P).transpose(2, 0, 1).reshape(P, 2 * n_dk))
    t = lambda w: np.ascontiguousarray(a(w).astype(np.float32).T.astype(bf))
    return {
        "WkT": t(inputs["Wk"]), "WvT": t(inputs["Wv"]),
        "WrT": t(-a(inputs["Wr"]).astype(np.float32)),
        "WoT": t(inputs["Wo"]), "FkT": t(inputs["Fk"]), "FvT": t(inputs["Fv"]),
        "FrT": t(inputs["Fr"]), "vecC": vecC_pm, "vecD": vecD_pm,
    }


_NC = None
LAST_EXEC_NS = None
LAST_RESULTS = None


def _get_nc():
    global _NC
    if _NC is None:
        nc = bacc.Bacc("TRN2", target_bir_lowering=False, debug=False)
        build_rwkv_kernel(nc)
        nc.compile()
        _NC = nc
    return _NC


def _maybe_install_trace_hook():
    """Best-effort NTFF profile hook shim (used when RWKV_BASS_TRACE=1)."""
    import types
    try:
        from antenv.axon_hooks import get_axon_ntff_profile_hook  # noqa: F401
        return True
    except ImportError:
        pass
    try:
        if "/root/.axon_site" not in sys.path and os.path.isdir("/root/.axon_site"):
            sys.path.insert(0, "/root/.axon_site")
        from trn_agent_boot.trn_boot import _ntff_profile_via_ctypes
        import antenv
        hookmod = types.ModuleType("antenv.axon_hooks")
        hookmod._hook = _ntff_profile_via_ctypes("/opt/axon/libaxon_pjrt.so")
        hookmod.set_axon_ntff_profile_hook = lambda h: setattr(hookmod, "_hook", h)
        hookmod.get_axon_ntff_profile_hook = lambda: hookmod._hook
        sys.modules["antenv.axon_hooks"] = hookmod
        antenv.axon_hooks = hookmod
        return True
    except Exception:
        return False


def kernel(**inputs):
    global LAST_EXEC_NS
    x = np.asarray(inputs["x"], dtype=np.float32)
    assert x.shape == (B, T, C), x.shape
    nc = _get_nc()
    shared = make_host_inputs(inputs)
    in_maps = [dict(shared, x=np.ascontiguousarray(x[i])) for i in range(N_CORES)]
    trace = os.environ.get("RWKV_BASS_TRACE", "") == "1"
    if trace:
        trace = _maybe_install_trace_hook()
    res = run_bass_kernel_spmd(nc, in_maps, list(range(N_CORES)), trace=trace)
    global LAST_RESULTS
    LAST_RESULTS = res
    LAST_EXEC_NS = res.exec_time_ns
    out = np.stack([res.results[i]["out"] for i in range(N_CORES)])
    return out.astype(np.float32)


# revision 13
# speedup vs baseline: 1.0989x; 1.0540x over previous
"""RWKV-4 block (TimeMix + ChannelMix) on 8 Trainium2 NeuronCores.

Sharding: data-parallel over batch (B=8 -> one batch element per core); no
collectives.  Per core, activations are kept transposed ([channel, time]) so
the WKV recurrence maps onto the DVE's hardware linear scan
(tensor_tensor_scan along the free axis, fp32 state) and channel-wise mix
coefficients become per-partition scalars.  LayerNorms run in the natural
[time, channel] layout; PE transposes move between the two.  All GEMMs run
in bf16 (full PE rate, overlapped LDWEIGHTS); WKV state in fp32.

WKV in direct form with the receptance sigmoid folded into the division:
  lam = exp(-exp(time_decay)), eu = exp(time_first)      (host)
  A_t = lam*A_{t-1} + exp(k_t)*v_t ;  B_t likewise with exp(k_t)
  y_t*sr_t = (A_{t-1} + eu*ek_t*v_t) / ((B_{t-1} + eu*ek_t)*(1+exp(-r_t)))
Wr is negated host-side so exp(-r) is a plain Exp on the ACT engine; the
whole phase then needs only the exp activation table (plus one rsqrt load
for LayerNorm), and the scans run over the full T=2048 with no carries.

v3 layout: LN hoisted for all of T, mixes computed once at [P, 2048]
(bf16), per-dk GEMM chunks feed full-length scans, elementwise work spread
DVE/GpSimd/ACT (GpSimd only does bf16/f32xbf16 tensor_tensor - other op
shapes miscompile), Fr+sigmoid folded into the Wo phase, Fk/Fv phase kept
PE-dense, weights staged across phase boundaries on the GpSimd DMA queue.
"""

import os
import sys
from contextlib import ExitStack

import numpy as np

for _p in ("/opt/trn_rl_repo", "/root/.axon_site/_ro/trn_rl_repo"):
    if os.path.isdir(_p) and _p not in sys.path:
        sys.path.insert(0, _p)
        break

import concourse.bass as bass
import concourse.tile as tile
from concourse import mybir, bacc
from concourse.bass_utils import run_bass_kernel_spmd
from concourse.masks import make_identity

f32 = mybir.dt.float32
bf16 = mybir.dt.bfloat16
AF = mybir.ActivationFunctionType
ALU = mybir.AluOpType
P = 128
EPS = 1e-5
ts = bass.ts

B, T, C, DA, DF = 8, 2048, 1024, 1024, 4096
N_CORES = 8


def build_rwkv_kernel(nc, T=T, C=C, DA=DA, DF=DF):
    TT = 1024            # Wo/LN2 tile (time)
    TC = 512             # ChannelMix tile (time)
    n_t = T // TT        # 2
    n_tc = T // TC       # 4
    n_ck = C // P        # 8
    n_dk = DA // P       # 8
    n_fk = DF // P       # 32
    n_rs = TT // P       # 8 row-tiles per TT tile
    n_rc = TC // P       # 4 row-tiles per TC tile
    n_q = T // 512       # 4 GEMM chunks over full T
    fph = n_fk // 2      # 16 f-tiles per half

    dma = nc.sync.dma_start

    x_d = nc.dram_tensor("x", [T, C], f32, kind="ExternalInput")
    wkT_d = nc.dram_tensor("WkT", [C, DA], bf16, kind="ExternalInput")
    wvT_d = nc.dram_tensor("WvT", [C, DA], bf16, kind="ExternalInput")
    wrT_d = nc.dram_tensor("WrT", [C, DA], bf16, kind="ExternalInput")
    woT_d = nc.dram_tensor("WoT", [DA, C], bf16, kind="ExternalInput")
    fkT_d = nc.dram_tensor("FkT", [C, DF], bf16, kind="ExternalInput")
    fvT_d = nc.dram_tensor("FvT", [DF, C], bf16, kind="ExternalInput")
    frT_d = nc.dram_tensor("FrT", [C, C], bf16, kind="ExternalInput")
    vc_d = nc.dram_tensor("vecC", [P, 9 * n_ck], f32, kind="ExternalInput")
    vd_d = nc.dram_tensor("vecD", [P, 2 * n_dk], f32, kind="ExternalInput")
    out_d = nc.dram_tensor("out", [T, C], f32, kind="ExternalOutput")

    with tile.TileContext(nc) as tc, ExitStack() as top:
        const = top.enter_context(tc.tile_pool(name="const", bufs=1))
        vc = const.tile([P, 9, n_ck], f32)
        dma(out=vc, in_=vc_d[:].rearrange("p (r a) -> p r a", a=n_ck))
        vd = const.tile([P, 2, n_dk], f32)
        dma(out=vd, in_=vd_d[:].rearrange("p (r a) -> p r a", a=n_dk))
        V = {
            "ln1_g": lambda ck: vc[:, 0, ck:ck + 1],
            "ln1_b": lambda ck: vc[:, 1, ck:ck + 1],
            "ln2_g": lambda ck: vc[:, 2, ck:ck + 1],
            "ln2_b": lambda ck: vc[:, 3, ck:ck + 1],
            "tm_k": lambda ck: vc[:, 4, ck:ck + 1],
            "tm_v": lambda ck: vc[:, 5, ck:ck + 1],
            "tm_r": lambda ck: vc[:, 6, ck:ck + 1],
            "fm_k": lambda ck: vc[:, 7, ck:ck + 1],
            "fm_r": lambda ck: vc[:, 8, ck:ck + 1],
            "lam": lambda dk: vd[:, 0, dk:dk + 1],
            "eu": lambda dk: vd[:, 1, dk:dk + 1],
        }
        ident_b = const.tile([P, P], bf16)
        make_identity(nc, ident_b)
        eps_t = const.tile([P, 1], f32)
        nc.vector.memset(eps_t, EPS)
        one_t = const.tile([P, 1], f32)
        nc.vector.memset(one_t, 1.0)

        dp_rw = top.enter_context(
            tc.tile_pool(name="dp_rw", bufs=n_dk * n_t, space="DRAM"))
        dp_gk = top.enter_context(
            tc.tile_pool(name="dp_gk", bufs=n_ck * n_t, space="DRAM"))
        dp_sg = top.enter_context(
            tc.tile_pool(name="dp_sg", bufs=n_ck * n_t, space="DRAM"))
        dp_o1 = top.enter_context(
            tc.tile_pool(name="dp_o1", bufs=T // P, space="DRAM"))
        rw_dr, gk_dr, sg_dr, o1_dr = {}, {}, {}, {}

        def layernorm(pool, tagp, xr, n_chunk=2):
            st = pool.tile([P, n_chunk, 6], f32, tag=f"{tagp}_st",
                           name=f"{tagp}_st")
            cw = C // n_chunk
            for j in range(n_chunk):
                nc.vector.bn_stats(out=st[:, j, :], in_=xr[:, ts(j, cw)])
            mv = pool.tile([P, 2], f32, tag=f"{tagp}_mv", name=f"{tagp}_mv")
            nc.vector.bn_aggr(out=mv, in_=st)
            rstd = pool.tile([P, 1], f32, tag=f"{tagp}_rstd",
                             name=f"{tagp}_rstd")
            nc.scalar.activation(out=rstd, in_=mv[:, 1:2],
                                 func=AF.Abs_reciprocal_sqrt,
                                 bias=eps_t[:, 0:1])
            nbias = pool.tile([P, 1], f32, tag=f"{tagp}_nb", name=f"{tagp}_nb")
            nc.vector.tensor_tensor(out=nbias, in0=mv[:, 0:1], in1=rstd,
                                    op=ALU.mult)
            nc.vector.tensor_scalar_mul(out=nbias, in0=nbias, scalar1=-1.0)
            return rstd, nbias

        # ---------------- Phase AB1: LN1 + mix + k/v/r GEMMs + WKV --------
        # Pipelined per half (TT=1024): LN(h) -> mixes(h) -> WKV(h), with
        # LN/mix of h1 overlapping WKV of h0.  Scan state carried across
        # halves through carryA/carryB (tiny DVE copies).
        with ExitStack() as ctx:
            wp = ctx.enter_context(tc.tile_pool(name="ab1_w", bufs=1))
            wk_sb = wp.tile([P, n_ck, DA], bf16)
            wv_sb = wp.tile([P, n_ck, DA], bf16)
            wr_sb = wp.tile([P, n_ck, DA], bf16)
            dma(out=wk_sb, in_=wkT_d[:].rearrange("(a p) d -> p a d", p=P))
            dma(out=wv_sb, in_=wvT_d[:].rearrange("(a p) d -> p a d", p=P))
            dma(out=wr_sb, in_=wrT_d[:].rearrange("(a p) d -> p a d", p=P))

            carry = ctx.enter_context(tc.tile_pool(name="ab1_carry", bufs=1))
            carryA = carry.tile([P, n_dk], f32)
            carryB = carry.tile([P, n_dk], f32)

            hp = ctx.enter_context(tc.tile_pool(name="ab1_h", bufs=1))
            mxp = ctx.enter_context(tc.tile_pool(name="ab1_mx", bufs=1))
            lnp = ctx.enter_context(tc.tile_pool(name="ab1_ln", bufs=2))
            xp = ctx.enter_context(tc.tile_pool(name="ab1_x", bufs=2))
            yp = ctx.enter_context(tc.tile_pool(name="ab1_y", bufs=1))
            dpool = ctx.enter_context(tc.tile_pool(name="ab1_d", bufs=1))
            wkv = ctx.enter_context(tc.tile_pool(name="ab1_wkv", bufs=1))
            ps_tr = ctx.enter_context(
                tc.tile_pool(name="ab1_ps_tr", bufs=2, space="PSUM"))
            ps_kv = ctx.enter_context(
                tc.tile_pool(name="ab1_ps_kv", bufs=4, space="PSUM"))
            ps_r = ctx.enter_context(
                tc.tile_pool(name="ab1_ps_r", bufs=2, space="PSUM"))

            hT = [hp.tile([P, 1 + T], bf16, tag=f"hT{ck}", name=f"hT{ck}")
                  for ck in range(n_ck)]
            for ck in range(n_ck):
                nc.vector.memset(hT[ck][:, 0:1], 0.0)

            for half in range(n_t):
                t0 = half * TT
                # --- LN over this half ---
                ys = []
                for rs in range(n_rs):
                    row = half * n_rs + rs
                    xr = xp.tile([P, C], f32, tag="xr1", name="xr1")
                    dma(out=xr, in_=x_d[ts(row, P), :])
                    rstd, nbias = layernorm(lnp, "l1", xr)
                    y = yp.tile([P, C], bf16, tag=f"y{rs}", name=f"y{rs}")
                    nc.scalar.activation(out=y, in_=xr, func=AF.Identity,
                                         bias=nbias[:, 0:1],
                                         scale=rstd[:, 0:1])
                    ys.append(y)
                for ck in range(n_ck):
                    pt = ps_tr.tile([P, TT], bf16, tag="pt", name="pt")
                    for rs in range(n_rs):
                        nc.tensor.transpose(pt[:, ts(rs, P)],
                                            ys[rs][:, ts(ck, P)], ident_b)
                    nc.scalar.activation(
                        out=hT[ck][:, 1 + t0:1 + t0 + TT], in_=pt,
                        func=AF.Identity, bias=V["ln1_b"](ck),
                        scale=V["ln1_g"](ck))

                # --- mixes for this half (d split DVE/GpSimd by parity) ---
                mixes = {}
                for ck in range(n_ck):
                    cur = hT[ck][:, 1 + t0:1 + t0 + TT]
                    prv = hT[ck][:, t0:t0 + TT]
                    d = dpool.tile([P, TT], bf16, tag="dmix", name="dmix",
                                   bufs=3)
                    eng = nc.gpsimd if ck % 2 == 0 else nc.vector
                    eng.tensor_tensor(out=d, in0=cur, in1=prv,
                                      op=ALU.subtract)
                    for nm, coef in (("xk", "tm_k"), ("xv", "tm_v"),
                                     ("xr", "tm_r")):
                        td = dpool.tile([P, TT], bf16, tag="tmx", name="tmx",
                                        bufs=2)
                        nc.scalar.activation(out=td, in_=d,
                                             func=AF.Identity,
                                             scale=V[coef](ck))
                        mx = mxp.tile([P, TT], bf16, tag=f"{nm}{ck}",
                                      name=f"{nm}{ck}")
                        nc.vector.tensor_tensor(out=mx, in0=td, in1=prv,
                                                op=ALU.add)
                        mixes[(nm, ck)] = mx

                # --- WKV per dk over this half ---
                for dk in range(n_dk):
                    ek = wkv.tile([P, TT], bf16, tag="ek", name="ek", bufs=2)
                    em = wkv.tile([P, TT], bf16, tag="em", name="em", bufs=1)
                    vsb = wkv.tile([P, TT], bf16, tag="vsb", name="vsb",
                                   bufs=2)
                    ekv = wkv.tile([P, TT], bf16, tag="ekv", name="ekv",
                                   bufs=2)
                    for q in range(TT // 512):
                        qs = ts(q, 512)
                        pk = ps_kv.tile([P, 512], f32, tag="pkv", name="pkv")
                        for ck in range(n_ck):
                            nc.tensor.matmul(pk, wk_sb[:, ck, ts(dk, P)],
                                             mixes[("xk", ck)][:, qs],
                                             start=(ck == 0),
                                             stop=(ck == n_ck - 1))
                        nc.scalar.activation(out=ek[:, qs], in_=pk,
                                             func=AF.Exp)
                        pv = ps_kv.tile([P, 512], f32, tag="pkv", name="pkv")
                        for ck in range(n_ck):
                            nc.tensor.matmul(pv, wv_sb[:, ck, ts(dk, P)],
                                             mixes[("xv", ck)][:, qs],
                                             start=(ck == 0),
                                             stop=(ck == n_ck - 1))
                        nc.scalar.copy(out=vsb[:, qs], in_=pv)
                        nc.gpsimd.tensor_tensor(out=ekv[:, qs],
                                                in0=ek[:, qs],
                                                in1=vsb[:, qs], op=ALU.mult)
                        pr = ps_r.tile([P, 512], f32, tag="pr", name="pr")
                        for ck in range(n_ck):
                            # Wr is negated host-side: pr = -r
                            nc.tensor.matmul(pr, wr_sb[:, ck, ts(dk, P)],
                                             mixes[("xr", ck)][:, qs],
                                             start=(ck == 0),
                                             stop=(ck == n_ck - 1))
                        nc.scalar.activation(out=em[:, qs], in_=pr,
                                             func=AF.Exp)
                    # ep = 1 + exp(-r)
                    ep = wkv.tile([P, TT], bf16, tag="ep", name="ep", bufs=1)
                    nc.scalar.activation(out=ep, in_=em, func=AF.Identity,
                                         bias=one_t[:, 0:1])

                    A = wkv.tile([P, 1 + TT], f32, tag="A", name="A", bufs=1)
                    Bt = wkv.tile([P, 1 + TT], f32, tag="B", name="B",
                                  bufs=1)
                    lam_b = V["lam"](dk).to_broadcast([P, TT])
                    if half == 0:
                        nc.vector.memset(A[:, 0:1], 0.0)
                        nc.vector.memset(Bt[:, 0:1], 0.0)
                    else:
                        nc.vector.tensor_copy(out=A[:, 0:1],
                                              in_=carryA[:, dk:dk + 1])
                        nc.vector.tensor_copy(out=Bt[:, 0:1],
                                              in_=carryB[:, dk:dk + 1])
                    nc.vector.tensor_tensor_scan(
                        out=A[:, 1:1 + TT], data0=lam_b, data1=ekv,
                        initial=A[:, 0:1], op0=ALU.mult, op1=ALU.add)
                    nc.vector.tensor_tensor_scan(
                        out=Bt[:, 1:1 + TT], data0=lam_b, data1=ek,
                        initial=Bt[:, 0:1], op0=ALU.mult, op1=ALU.add)
                    if half != n_t - 1:
                        nc.vector.tensor_copy(out=carryA[:, dk:dk + 1],
                                              in_=A[:, TT:TT + 1])
                        nc.vector.tensor_copy(out=carryB[:, dk:dk + 1],
                                              in_=Bt[:, TT:TT + 1])

                    num = wkv.tile([P, TT], bf16, tag="num", name="num",
                                   bufs=1)
                    nc.vector.scalar_tensor_tensor(
                        out=num, in0=ekv, scalar=V["eu"](dk),
                        in1=A[:, 0:TT], op0=ALU.mult, op1=ALU.add)
                    den = wkv.tile([P, TT], f32, tag="den", name="den",
                                   bufs=1)
                    nc.vector.scalar_tensor_tensor(
                        out=den, in0=ek, scalar=V["eu"](dk),
                        in1=Bt[:, 0:TT], op0=ALU.mult, op1=ALU.add)
                    den2 = wkv.tile([P, TT], f32, tag="den2", name="den2",
                                    bufs=1)
                    nc.gpsimd.tensor_tensor(out=den2, in0=den, in1=ep,
                                            op=ALU.mult)
                    rec = wkv.tile([P, TT], f32, tag="rec", name="rec",
                                   bufs=1)
                    nc.vector.reciprocal_approx_fast(out=rec, in_=den2)
                    rw = wkv.tile([P, TT], bf16, tag="rw", name="rw", bufs=1)
                    nc.vector.tensor_tensor(out=rw, in0=num, in1=rec,
                                            op=ALU.mult)
                    rd = dp_rw.tile([P, TT], bf16, tag="rw_dr", name="rw_dr")
                    dma(out=rd, in_=rw)
                    rw_dr[(dk, half)] = rd

        # Fk first half prefetch: overlaps AB2 compute, survives into CD
        fkp = top.enter_context(tc.tile_pool(name="fk_a", bufs=1))
        fka = fkp.tile([P, n_ck, DF // 2], bf16)

        # ---------------- Phase AB2: Wo + LN2 + gmix + Fr/sigmoid ---------
        with ExitStack() as ctx:
            wp2 = ctx.enter_context(tc.tile_pool(name="ab2_w", bufs=1))
            wo_sb = wp2.tile([P, n_dk, C], bf16)
            dma(out=wo_sb, in_=woT_d[:].rearrange("(a p) c -> p a c", p=P))
            fr_sb = wp2.tile([P, n_ck, C], bf16)
            nc.gpsimd.dma_start(
                out=fr_sb, in_=frT_d[:].rearrange("(a p) c -> p a c", p=P))
            nc.gpsimd.dma_start(
                out=fka,
                in_=fkT_d[:, 0:DF // 2].rearrange("(a p) d -> p a d", p=P))

            rwp = ctx.enter_context(tc.tile_pool(name="ab2_rw", bufs=1))
            xop = ctx.enter_context(tc.tile_pool(name="ab2_xo", bufs=1))
            xp2 = ctx.enter_context(tc.tile_pool(name="ab2_x", bufs=2))
            o1p = ctx.enter_context(tc.tile_pool(name="ab2_o1", bufs=2))
            y2p = ctx.enter_context(tc.tile_pool(name="ab2_y2", bufs=1))
            gp = ctx.enter_context(tc.tile_pool(name="ab2_g", bufs=1))
            g2p = ctx.enter_context(tc.tile_pool(name="ab2_g2", bufs=1))
            sgp = ctx.enter_context(tc.tile_pool(name="ab2_sg", bufs=2))
            lnp2 = ctx.enter_context(tc.tile_pool(name="ab2_ln", bufs=2))
            ps_o = ctx.enter_context(
                tc.tile_pool(name="ab2_ps_o", bufs=2, space="PSUM"))
            ps_so = ctx.enter_context(
                tc.tile_pool(name="ab2_ps_so", bufs=2, space="PSUM"))
            ps_g2 = ctx.enter_context(
                tc.tile_pool(name="ab2_ps_g2", bufs=2, space="PSUM"))
            ps_rr = ctx.enter_context(
                tc.tile_pool(name="ab2_ps_rr", bufs=2, space="PSUM"))

            gT = [gp.tile([P, 1 + T], bf16, tag=f"gT{ck}", name=f"gT{ck}")
                  for ck in range(n_ck)]

            for it in range(n_t):
                t0 = it * TT
                rws = []
                for dk in range(n_dk):
                    r = rwp.tile([P, TT], bf16, tag=f"rw2_{dk}",
                                 name=f"rw2_{dk}")
                    dma(out=r, in_=rw_dr[(dk, it)])
                    rws.append(r)
                xos = []
                for ck in range(n_ck):
                    xo = xop.tile([P, TT], bf16, tag=f"xo{ck}", name=f"xo{ck}")
                    for hh in range(TT // 512):
                        hs = ts(hh, 512)
                        po = ps_o.tile([P, 512], f32, tag="po", name="po")
                        for dk in range(n_dk):
                            nc.tensor.matmul(po, wo_sb[:, dk, ts(ck, P)],
                                             rws[dk][:, hs],
                                             start=(dk == 0),
                                             stop=(dk == n_dk - 1))
                        nc.scalar.copy(out=xo[:, hs], in_=po)
                    xos.append(xo)
                y2s = []
                for rs in range(n_rs):
                    row = it * n_rs + rs
                    pso = ps_so.tile([P, C], bf16, tag="pso", name="pso")
                    for ck in range(n_ck):
                        nc.tensor.transpose(pso[:, ts(ck, P)],
                                            xos[ck][:, ts(rs, P)], ident_b)
                    xr2 = xp2.tile([P, C], f32, tag="xr2", name="xr2")
                    dma(out=xr2, in_=x_d[ts(row, P), :])
                    o1 = o1p.tile([P, C], bf16, tag="o1", name="o1")
                    nc.vector.tensor_tensor(out=o1, in0=xr2, in1=pso,
                                            op=ALU.add)
                    od = dp_o1.tile([P, C], bf16, tag="o1_dr", name="o1_dr")
                    dma(out=od, in_=o1)
                    o1_dr[row] = od
                    rstd, nbias = layernorm(lnp2, "l2", o1)
                    y2 = y2p.tile([P, C], bf16, tag=f"y2_{rs}",
                                  name=f"y2_{rs}")
                    nc.scalar.activation(out=y2, in_=o1, func=AF.Identity,
                                         bias=nbias[:, 0:1], scale=rstd[:, 0:1])
                    y2s.append(y2)
                for ck in range(n_ck):
                    pg = ps_g2.tile([P, TT], bf16, tag="pg", name="pg")
                    for rs in range(n_rs):
                        nc.tensor.transpose(pg[:, ts(rs, P)],
                                            y2s[rs][:, ts(ck, P)], ident_b)
                    if it == 0:
                        nc.vector.memset(gT[ck][:, 0:1], 0.0)
                    nc.scalar.activation(out=gT[ck][:, 1 + t0:1 + t0 + TT],
                                         in_=pg, func=AF.Identity,
                                         bias=V["ln2_b"](ck),
                                         scale=V["ln2_g"](ck))
                grs = []
                for ck in range(n_ck):
                    cur = gT[ck][:, 1 + t0:1 + t0 + TT]
                    prv = gT[ck][:, t0:t0 + TT]
                    d2 = g2p.tile([P, TT], bf16, tag="d2", name="d2", bufs=2)
                    nc.vector.tensor_tensor(out=d2, in0=cur, in1=prv,
                                            op=ALU.subtract)
                    tg = g2p.tile([P, TT], bf16, tag="tgk", name="tgk", bufs=2)
                    nc.scalar.activation(out=tg, in_=d2, func=AF.Identity,
                                         scale=V["fm_k"](ck))
                    gk = g2p.tile([P, TT], bf16, tag="gkm", name="gkm", bufs=2)
                    nc.vector.tensor_tensor(out=gk, in0=tg, in1=prv,
                                            op=ALU.add)
                    gkd = dp_gk.tile([P, TT], bf16, tag="gk_dr", name="gk_dr")
                    dma(out=gkd, in_=gk)
                    gk_dr[(ck, it)] = gkd
                    gr = g2p.tile([P, TT], bf16, tag=f"gr{ck}", name=f"gr{ck}")
                    nc.vector.scalar_tensor_tensor(
                        out=gr, in0=d2, scalar=V["fm_r"](ck), in1=prv,
                        op0=ALU.mult, op1=ALU.add)
                    grs.append(gr)
                for ck in range(n_ck):
                    sg = sgp.tile([P, TT], bf16, tag="sg", name="sg")
                    for hh in range(TT // 512):
                        hs = ts(hh, 512)
                        prr = ps_rr.tile([P, 512], f32, tag="prr", name="prr")
                        for cj in range(n_ck):
                            nc.tensor.matmul(prr, fr_sb[:, cj, ts(ck, P)],
                                             grs[cj][:, hs],
                                             start=(cj == 0),
                                             stop=(cj == n_ck - 1))
                        nc.scalar.activation(out=sg[:, hs], in_=prr,
                                             func=AF.Sigmoid)
                    sgd = dp_sg.tile([P, TT], bf16, tag="sg_dr", name="sg_dr")
                    dma(out=sgd, in_=sg)
                    sg_dr[(ck, it)] = sgd

        # ---------------- Phase CD: Fk relu^2, Fv, combine + out ----------
        with ExitStack() as ctx:
            wp3 = ctx.enter_context(tc.tile_pool(name="cd_w", bufs=1))
            fkb = wp3.tile([P, n_ck, DF // 2], bf16)
            nc.gpsimd.dma_start(
                out=fkb,
                in_=fkT_d[:, DF // 2:].rearrange("(a p) d -> p a d", p=P))
            fv_sb = wp3.tile([P, n_fk, C], bf16)
            dma(out=fv_sb, in_=fvT_d[:].rearrange("(a p) c -> p a c", p=P))

            def fk_ap(fk):
                if fk < fph:
                    return lambda ck: fka[:, ck, ts(fk, P)]
                return lambda ck: fkb[:, ck, ts(fk - fph, P)]

            gkc = ctx.enter_context(tc.tile_pool(name="cd_gk", bufs=1))
            kfp = ctx.enter_context(tc.tile_pool(name="cd_kf", bufs=1))
            cp = ctx.enter_context(tc.tile_pool(name="cd_cp", bufs=2))
            kvp = ctx.enter_context(tc.tile_pool(name="cd_kv", bufs=1))
            prodp = ctx.enter_context(tc.tile_pool(name="cd_prod", bufs=1))
            finp = ctx.enter_context(tc.tile_pool(name="cd_fin", bufs=2))
            ps_kf = ctx.enter_context(
                tc.tile_pool(name="cd_ps_kf", bufs=2, space="PSUM"))
            ps_kvp = ctx.enter_context(
                tc.tile_pool(name="cd_ps_kv", bufs=2, space="PSUM"))
            ps_sp = ctx.enter_context(
                tc.tile_pool(name="cd_ps_sp", bufs=2, space="PSUM"))

            for itc in range(n_tc):
                it2, h2 = itc // (n_tc // n_t), itc % (n_tc // n_t)
                hs2 = ts(h2, TC)
                gks = []
                for ck in range(n_ck):
                    gk = gkc.tile([P, TC], bf16, tag=f"gkc{ck}",
                                  name=f"gkc{ck}")
                    dma(out=gk, in_=gk_dr[(ck, it2)][:, hs2])
                    gks.append(gk)
                kv0 = {}
                kvs = {}
                for hf in range(2):
                    kf = kfp.tile([P, fph, TC], bf16, tag="kf", name="kf")
                    for fj in range(fph):
                        fk = hf * fph + fj
                        wap = fk_ap(fk)
                        pkf = ps_kf.tile([P, TC], f32, tag="pkf", name="pkf")
                        for ck in range(n_ck):
                            nc.tensor.matmul(pkf, wap(ck), gks[ck],
                                             start=(ck == 0),
                                             stop=(ck == n_ck - 1))
                        r1 = cp.tile([P, TC], bf16, tag="r1", name="r1")
                        nc.scalar.activation(out=r1, in_=pkf, func=AF.Relu)
                        nc.vector.tensor_tensor(out=kf[:, fj, :], in0=r1,
                                                in1=r1, op=ALU.mult)
                    for ck in range(n_ck):
                        pkv = ps_kvp.tile([P, TC], f32, tag="pkv", name="pkv")
                        for fj in range(fph):
                            nc.tensor.matmul(pkv,
                                             fv_sb[:, hf * fph + fj, ts(ck, P)],
                                             kf[:, fj, :],
                                             start=(fj == 0),
                                             stop=(fj == fph - 1))
                        if hf == 0:
                            k0 = kvp.tile([P, TC], bf16, tag=f"kv0_{ck}",
                                          name=f"kv0_{ck}")
                            nc.scalar.copy(out=k0, in_=pkv)
                            kv0[ck] = k0
                        else:
                            kv = kvp.tile([P, TC], bf16, tag=f"kv_{ck}",
                                          name=f"kv_{ck}")
                            nc.vector.tensor_tensor(out=kv, in0=kv0[ck],
                                                    in1=pkv, op=ALU.add)
                            kvs[ck] = kv
                prods = []
                for ck in range(n_ck):
                    sgt = cp.tile([P, TC], bf16, tag="sgl", name="sgl")
                    dma(out=sgt, in_=sg_dr[(ck, it2)][:, hs2])
                    prod = prodp.tile([P, TC], bf16, tag=f"prod{ck}",
                                      name=f"prod{ck}")
                    nc.vector.tensor_tensor(out=prod, in0=sgt, in1=kvs[ck],
                                            op=ALU.mult)
                    prods.append(prod)
                for rs in range(n_rc):
                    row = itc * n_rc + rs
                    psp = ps_sp.tile([P, C], bf16, tag="psp", name="psp")
                    for ck in range(n_ck):
                        nc.tensor.transpose(psp[:, ts(ck, P)],
                                            prods[ck][:, ts(rs, P)], ident_b)
                    o1t = finp.tile([P, C], bf16, tag="o1c", name="o1c")
                    dma(out=o1t, in_=o1_dr[row])
                    fin = finp.tile([P, C], f32, tag="fin", name="fin")
                    nc.vector.tensor_tensor(out=fin, in0=o1t, in1=psp,
                                            op=ALU.add)
                    dma(out=out_d[ts(row, P), :], in_=fin)
    return nc


def make_host_inputs(inputs, C=C, DA=DA):
    import ml_dtypes
    bf = ml_dtypes.bfloat16
    a = np.asarray
    n_ck = C // P
    n_dk = DA // P
    vecC = np.stack([
        a(inputs["ln1_g"]), a(inputs["ln1_b"]),
        a(inputs["ln2_g"]), a(inputs["ln2_b"]),
        a(inputs["tm_k"]), a(inputs["tm_v"]), a(inputs["tm_r"]),
        a(inputs["fm_k"]), a(inputs["fm_r"]),
    ]).astype(np.float32)
    vecD = np.stack([
        np.exp(-np.exp(a(inputs["time_decay"]).astype(np.float64))),
        np.exp(a(inputs["time_first"]).astype(np.float64)),
    ]).astype(np.float32)
    vecC_pm = np.ascontiguousarray(
        vecC.reshape(9, n_ck, P).transpose(2, 0, 1).reshape(P, 9 * n_ck))
    vecD_pm = np.ascontiguousarray(
        vecD.reshape(2, n_dk, P).transpose(2, 0, 1).reshape(P, 2 * n_dk))
    t = lambda w: np.ascontiguousarray(a(w).astype(np.float32).T.astype(bf))
    return {
        "WkT": t(inputs["Wk"]), "WvT": t(inputs["Wv"]),
        "WrT": t(-a(inputs["Wr"]).astype(np.float32)),
        "WoT": t(inputs["Wo"]), "FkT": t(inputs["Fk"]), "FvT": t(inputs["Fv"]),
        "FrT": t(inputs["Fr"]), "vecC": vecC_pm, "vecD": vecD_pm,
    }


_NC = None
LAST_EXEC_NS = None
LAST_RESULTS = None


def _get_nc():
    global _NC
    if _NC is None:
        nc = bacc.Bacc("TRN2", target_bir_lowering=False, debug=False)
        build_rwkv_kernel(nc)
        nc.compile()
        _NC = nc
    return _NC


def _maybe_install_trace_hook():
    """Best-effort NTFF profile hook shim (used when RWKV_BASS_TRACE=1)."""
    import types
    try:
        from antenv.axon_hooks import get_axon_ntff_profile_hook  # noqa: F401
        return True
    except ImportError:
        pass
    try:
        if "/root/.axon_site" not in sys.path and os.path.isdir("/root/.axon_site"):
            sys.path.insert(0, "/root/.axon_site")
        from trn_agent_boot.trn_boot import _ntff_profile_via_ctypes
        import antenv
        hookmod = types.ModuleType("antenv.axon_hooks")
        hookmod._hook = _ntff_profile_via_ctypes("/opt/axon/libaxon_pjrt.so")
        hookmod.set_axon_ntff_profile_hook = lambda h: setattr(hookmod, "_hook", h)
        hookmod.get_axon_ntff_profile_hook = lambda: hookmod._hook
        sys.modules["antenv.axon_hooks"] = hookmod
        antenv.axon_hooks = hookmod
        return True
    except Exception:
        return False


def kernel(**inputs):
    global LAST_EXEC_NS
    x = np.asarray(inputs["x"], dtype=np.float32)
    assert x.shape == (B, T, C), x.shape
    nc = _get_nc()
    shared = make_host_inputs(inputs)
    in_maps = [dict(shared, x=np.ascontiguousarray(x[i])) for i in range(N_CORES)]
    trace = os.environ.get("RWKV_BASS_TRACE", "") == "1"
    if trace:
        trace = _maybe_install_trace_hook()
    res = run_bass_kernel_spmd(nc, in_maps, list(range(N_CORES)), trace=trace)
    global LAST_RESULTS
    LAST_RESULTS = res
    LAST_EXEC_NS = res.exec_time_ns
    out = np.stack([res.results[i]["out"] for i in range(N_CORES)])
    return out.astype(np.float32)
